# revision 2
# baseline (speedup 1.0000x reference)
"""BsPINN forward MLP on 8 Trainium2 NeuronCores (data-parallel over batch).

Network (reference): X[65536,2] -> normalize -> tanh(XW0+b0) -> tanh(hW1+b1)
  -> tanh(h(W2*mask2)+b2) -> tanh(h(W3*mask3)+b3) -> hW_last+b_last -> [65536,1]
mask2 = blockdiag(2 x [512x512] ones), mask3 = blockdiag(4 x [256x256] ones).

Device strategy (per core, 8192 rows):
  - Activations kept TRANSPOSED in SBUF: [features(partitions), rows(free)].
    Every layer is psum[m] += W[kblk,mblk].T @ hT[kblk] via nc.tensor.matmul
    (out = lhsT.T @ rhs), so no transposes are ever needed on device.
  - Input normalization is folded into W0/b0 on the host.
  - Masked layers are packed on the host to only their nonzero 128x128 blocks
    (layer2: 4 k-blocks per m-tile, layer3: 2) -> 240.5 GFLOP total instead
    of 412.3 dense.
  - Matmuls run as float32r (full-rate on the PE at N=512); bias+tanh fused
    into one ScalarE activation per [128, ROWS_T] psum tile.
"""

import os

import numpy as np

N_CORES = 8
N_ROWS = 65536
R = N_ROWS // N_CORES  # rows per core
ROWS_T = 1024  # rows per row-tile (2 PSUM banks per m-tile)
N_RT = R // ROWS_T
H = 1024
P = 128
KT = H // P  # 8 feature tiles

_STATE = {}


def _build_module():
    import concourse.bacc as bacc
    import concourse.mybir as mybir
    import concourse.tile as tile

    f32 = mybir.dt.float32
    f32r = mybir.dt.float32r
    Tanh = mybir.ActivationFunctionType.Tanh

    nc = bacc.Bacc("TRN2", target_bir_lowering=False, debug=False)

    xT = nc.dram_tensor("xT", [2, R], f32r, kind="ExternalInput")
    w0 = nc.dram_tensor("w0", [2, H], f32r, kind="ExternalInput")
    w1 = nc.dram_tensor("w1", [H, H], f32r, kind="ExternalInput")
    w2 = nc.dram_tensor("w2", [H, 512], f32r, kind="ExternalInput")
    w3 = nc.dram_tensor("w3", [H, 256], f32r, kind="ExternalInput")
    wl = nc.dram_tensor("wl", [P, KT], f32r, kind="ExternalInput")
    bt = nc.dram_tensor("bt", [P, 4 * KT], f32, kind="ExternalInput")
    outT = nc.dram_tensor("outT", [1, R], f32, kind="ExternalOutput")

    with tile.TileContext(nc) as tc:
        with (
            tc.tile_pool(name="wpool", bufs=1) as wp,
            tc.tile_pool(name="hpool", bufs=1) as hp,
            tc.tile_pool(name="xpool", bufs=3) as xp,
            tc.tile_pool(name="opool", bufs=2) as op,
            tc.tile_pool(name="psum", bufs=3, space="PSUM") as pp,
            tc.tile_pool(name="psuml", bufs=2, space="PSUM") as pl,
        ):
            w0s = wp.tile([2, H], f32r, tag="w0")
            nc.sync.dma_start(w0s[:], w0[:])
            w1s = []
            w2s = []
            w3s = []
            for k in range(KT):
                t = wp.tile([P, H], f32r, tag=f"w1_{k}")
                nc.sync.dma_start(t[:], w1[k * P : (k + 1) * P, :])
                w1s.append(t)
            for k in range(KT):
                t = wp.tile([P, 512], f32r, tag=f"w2_{k}")
                nc.sync.dma_start(t[:], w2[k * P : (k + 1) * P, :])
                w2s.append(t)
            for k in range(KT):
                t = wp.tile([P, 256], f32r, tag=f"w3_{k}")
                nc.sync.dma_start(t[:], w3[k * P : (k + 1) * P, :])
                w3s.append(t)
            wls = wp.tile([P, KT], f32r, tag="wl")
            nc.sync.dma_start(wls[:], wl[:])
            bts = wp.tile([P, 4 * KT], f32, tag="bt")
            nc.sync.dma_start(bts[:], bt[:])

            for r in range(N_RT):
                rs = r * ROWS_T
                xs = xp.tile([2, ROWS_T], f32r, tag="x")
                nc.sync.dma_start(xs[:], xT[:, rs : rs + ROWS_T])

                # ---- layer 0: K=2 dense (normalization pre-folded) ----
                h0 = []
                for m in range(KT):
                    ps = pp.tile([P, ROWS_T], f32, tag="ps")
                    for hf in range(2):
                        c = hf * 512
                        nc.tensor.matmul(
                            ps[:, c : c + 512],
                            w0s[:, m * P : (m + 1) * P],
                            xs[:, c : c + 512],
                            start=True,
                            stop=True,
                        )
                    ht = hp.tile([P, ROWS_T], f32r, tag=f"hA_{m}")
                    nc.scalar.activation(ht[:], ps[:], Tanh, bias=bts[:, m : m + 1])
                    h0.append(ht)

                # ---- layer 1: dense 1024x1024 ----
                h1 = []
                for m in range(KT):
                    ps = pp.tile([P, ROWS_T], f32, tag="ps")
                    for hf in range(2):
                        c = hf * 512
                        for j, k in enumerate(range(KT)):
                            nc.tensor.matmul(
                                ps[:, c : c + 512],
                                w1s[k][:, m * P : (m + 1) * P],
                                h0[k][:, c : c + 512],
                                start=(j == 0),
                                stop=(j == KT - 1),
                            )
                    ht = hp.tile([P, ROWS_T], f32r, tag=f"hB_{m}")
                    nc.scalar.activation(
                        ht[:], ps[:], Tanh, bias=bts[:, KT + m : KT + m + 1]
                    )
                    h1.append(ht)

                # ---- layer 2: block-diagonal, 4 k-blocks per m-tile ----
                h2 = []
                for m in range(KT):
                    ks = [(m // 4) * 4 + j for j in range(4)]
                    mo = (m % 4) * P
                    ps = pp.tile([P, ROWS_T], f32, tag="ps")
                    for hf in range(2):
                        c = hf * 512
                        for j, k in enumerate(ks):
                            nc.tensor.matmul(
                                ps[:, c : c + 512],
                                w2s[k][:, mo : mo + P],
                                h1[k][:, c : c + 512],
                                start=(j == 0),
                                stop=(j == len(ks) - 1),
                            )
                    ht = hp.tile([P, ROWS_T], f32r, tag=f"hA_{m}")
                    nc.scalar.activation(
                        ht[:], ps[:], Tanh, bias=bts[:, 2 * KT + m : 2 * KT + m + 1]
                    )
                    h2.append(ht)

                # ---- layer 3: block-diagonal, 2 k-blocks per m-tile ----
                h3 = []
                for m in range(KT):
                    ks = [(m // 2) * 2 + j for j in range(2)]
                    mo = (m % 2) * P
                    ps = pp.tile([P, ROWS_T], f32, tag="ps")
                    for hf in range(2):
                        c = hf * 512
                        for j, k in enumerate(ks):
                            nc.tensor.matmul(
                                ps[:, c : c + 512],
                                w3s[k][:, mo : mo + P],
                                h2[k][:, c : c + 512],
                                start=(j == 0),
                                stop=(j == len(ks) - 1),
                            )
                    ht = hp.tile([P, ROWS_T], f32r, tag=f"hB_{m}")
                    nc.scalar.activation(
                        ht[:], ps[:], Tanh, bias=bts[:, 3 * KT + m : 3 * KT + m + 1]
                    )
                    h3.append(ht)

                # ---- output layer: [1, rows] ----
                ot = op.tile([1, ROWS_T], f32, tag="o")
                for hf in range(2):
                    c = hf * 512
                    psl = pl.tile([1, 512], f32, tag="psl")
                    for j, k in enumerate(range(KT)):
                        nc.tensor.matmul(
                            psl[:],
                            wls[:, k : k + 1],
                            h3[k][:, c : c + 512],
                            start=(j == 0),
                            stop=(j == KT - 1),
                        )
                    nc.vector.tensor_copy(ot[:, c : c + 512], psl[:])
                nc.sync.dma_start(outT[:, rs : rs + ROWS_T], ot[:])

    nc.compile()
    return nc


def _get_module():
    if "nc" not in _STATE:
        _STATE["nc"] = _build_module()
    return _STATE["nc"]


def _tcol(v):
    """[1024] vector -> [128, 8]: column t holds v[t*128:(t+1)*128]."""
    return np.ascontiguousarray(v.reshape(KT, P).T).astype(np.float32)


def prep_in_maps(inputs):
    X = np.asarray(inputs["X"], np.float32)
    W0 = np.asarray(inputs["W0"], np.float32)
    b0 = np.asarray(inputs["b0"], np.float32)
    W1 = np.asarray(inputs["W1"], np.float32)
    b1 = np.asarray(inputs["b1"], np.float32)
    W2 = np.asarray(inputs["W2"], np.float32)
    b2 = np.asarray(inputs["b2"], np.float32)
    W3 = np.asarray(inputs["W3"], np.float32)
    b3 = np.asarray(inputs["b3"], np.float32)
    Wl = np.asarray(inputs["W_last"], np.float32)

    # fold `h = 2*(X-LB)/(UB-LB) - 1` (LB=[0,0], UB=[2pi,1]) into layer 0
    s = np.array([1.0 / np.pi, 2.0], np.float32)
    w0p = np.ascontiguousarray(s[:, None] * W0)
    b0p = b0[0] - W0[0] - W0[1]

    # pack only the nonzero 128-blocks of the masked layers
    w2p = np.ascontiguousarray(
        np.concatenate(
            [W2[k * P : (k + 1) * P, (k // 4) * 512 : (k // 4) * 512 + 512] for k in range(KT)],
            axis=0,
        )
    )
    w3p = np.ascontiguousarray(
        np.concatenate(
            [W3[k * P : (k + 1) * P, (k // 2) * 256 : (k // 2) * 256 + 256] for k in range(KT)],
            axis=0,
        )
    )
    wlp = _tcol(Wl[:, 0])
    btp = np.ascontiguousarray(
        np.concatenate([_tcol(b0p), _tcol(b1[0]), _tcol(b2[0]), _tcol(b3[0])], axis=1)
    )

    xT = np.ascontiguousarray(X.T)  # [2, 65536]
    common = {"w0": w0p, "w1": W1, "w2": w2p, "w3": w3p, "wl": wlp, "bt": btp}
    return [
        {"xT": np.ascontiguousarray(xT[:, c * R : (c + 1) * R]), **common}
        for c in range(N_CORES)
    ]


def kernel(**inputs):
    from concourse.bass_utils import run_bass_kernel_spmd

    nc = _get_module()
    in_maps = prep_in_maps(inputs)
    trace = bool(int(os.environ.get("BASS_KERNEL_TRACE", "0")))
    res = run_bass_kernel_spmd(nc, in_maps, list(range(N_CORES)), trace=trace)
    _STATE["last_result"] = res
    out = np.concatenate([res.results[c]["outT"].reshape(-1) for c in range(N_CORES)])
    b_last = np.asarray(inputs["b_last"], np.float32)
    return (out.reshape(-1, 1) + b_last).astype(np.float32)


# revision 5
# speedup vs baseline: 1.0740x; 1.0740x over previous
"""BsPINN forward MLP on 8 Trainium2 NeuronCores (data-parallel over batch).

Network (reference): X[65536,2] -> normalize -> tanh(XW0+b0) -> tanh(hW1+b1)
  -> tanh(h(W2*mask2)+b2) -> tanh(h(W3*mask3)+b3) -> hW_last+b_last -> [65536,1]
mask2 = blockdiag(2 x [512x512] ones), mask3 = blockdiag(4 x [256x256] ones).

Device strategy (per core, 8192 rows):
  - Activations kept TRANSPOSED in SBUF: [features(partitions), rows(free)].
    Every layer is psum[m] += W[kblk,mblk].T @ hT[kblk] via nc.tensor.matmul
    (out = lhsT.T @ rhs), so no transposes are ever needed on device.
  - Input normalization is folded into W0/b0 on the host.
  - Masked layers are packed on the host to only their nonzero 128x128 blocks
    (layer2: 4 k-blocks per m-tile, layer3: 2) -> 240.5 GFLOP total instead
    of 412.3 dense.
  - Matmuls run as float32r (full-rate on the PE at N=512); bias+tanh fused
    into one ScalarE activation per [128, ROWS_T] psum tile.
"""

import os

import numpy as np

N_CORES = 8
N_ROWS = 65536
R = N_ROWS // N_CORES  # rows per core
ROWS_T = 1024  # rows per row-tile (2 PSUM banks per m-tile)
N_RT = R // ROWS_T
H = 1024
P = 128
KT = H // P  # 8 feature tiles

_STATE = {}


def _build_module():
    import concourse.bacc as bacc
    import concourse.mybir as mybir
    import concourse.tile as tile

    f32 = mybir.dt.float32
    f32r = mybir.dt.float32r
    Tanh = mybir.ActivationFunctionType.Tanh

    nc = bacc.Bacc("TRN2", target_bir_lowering=False, debug=False)

    xT = nc.dram_tensor("xT", [2, R], f32r, kind="ExternalInput")
    w0 = nc.dram_tensor("w0", [2, H], f32r, kind="ExternalInput")
    w1 = nc.dram_tensor("w1", [H, H], f32r, kind="ExternalInput")
    w2 = nc.dram_tensor("w2", [H, 512], f32r, kind="ExternalInput")
    w3 = nc.dram_tensor("w3", [H, 256], f32r, kind="ExternalInput")
    wl = nc.dram_tensor("wl", [P, KT], f32r, kind="ExternalInput")
    bt = nc.dram_tensor("bt", [P, 4 * KT], f32, kind="ExternalInput")
    outT = nc.dram_tensor("outT", [1, R], f32, kind="ExternalOutput")

    with tile.TileContext(nc) as tc:
        with (
            tc.tile_pool(name="wpool", bufs=1) as wp,
            tc.tile_pool(name="hpool", bufs=1) as hp,
            tc.tile_pool(name="xpool", bufs=2) as xp,
            tc.tile_pool(name="opool", bufs=2) as op,
            tc.tile_pool(name="psum", bufs=3, space="PSUM") as pp,
            tc.tile_pool(name="psuml", bufs=2, space="PSUM") as pl,
        ):
            w0s = wp.tile([2, H], f32r, tag="w0")
            bts = wp.tile([P, 4 * KT], f32, tag="bt")
            nc.sync.dma_start(w0s[:], w0[:])
            nc.sync.dma_start(bts[:], bt[:])
            xss = [None] * (N_RT + 1)

            def load_x(r):
                if r < N_RT:
                    xss[r] = xp.tile([2, ROWS_T], f32r, tag="x", name=f"xs{r}")
                    nc.sync.dma_start(xss[r][:], xT[:, r * ROWS_T : r * ROWS_T + ROWS_T])

            load_x(0)
            load_x(1)
            w1s = []
            w2s = []
            w3s = []
            for k in range(KT):
                t = wp.tile([P, H], f32r, tag=f"w1_{k}")
                nc.sync.dma_start(t[:], w1[k * P : (k + 1) * P, :])
                w1s.append(t)
            for k in range(KT):
                t = wp.tile([P, 512], f32r, tag=f"w2_{k}")
                nc.sync.dma_start(t[:], w2[k * P : (k + 1) * P, :])
                w2s.append(t)
            for k in range(KT):
                t = wp.tile([P, 256], f32r, tag=f"w3_{k}")
                nc.sync.dma_start(t[:], w3[k * P : (k + 1) * P, :])
                w3s.append(t)
            wls = wp.tile([P, KT], f32r, tag="wl")
            nc.sync.dma_start(wls[:], wl[:])

            hs = {}  # (layer, r) -> list of 8 tiles

            def dense_group(r, layer, m, wk, hin, hout_tag, ks, mo):
                """One m-tile group: matmuls over ks into a psum pair + tanh."""
                ps = pp.tile([P, ROWS_T], f32, tag="ps", name=f"ps{layer}_{m}")
                for hf in range(2):
                    c = hf * 512
                    for j, k in enumerate(ks):
                        nc.tensor.matmul(
                            ps[:, c : c + 512],
                            wk[k][:, mo(k) : mo(k) + P] if callable(mo) else wk[k][:, mo : mo + P],
                            hin[k][:, c : c + 512],
                            start=(j == 0),
                            stop=(j == len(ks) - 1),
                        )
                ht = hp.tile([P, ROWS_T], f32r, tag=f"{hout_tag}_{m}", name=f"{hout_tag}_{m}")
                nc.scalar.activation(
                    ht[:], ps[:], Tanh, bias=bts[:, layer * KT + m : layer * KT + m + 1]
                )
                return ht

            def l0_group(r, m):
                ps = pp.tile([P, ROWS_T], f32, tag="ps", name=f"ps0_{m}")
                for hf in range(2):
                    c = hf * 512
                    nc.tensor.matmul(
                        ps[:, c : c + 512],
                        w0s[:, m * P : (m + 1) * P],
                        xss[r][:, c : c + 512],
                        start=True,
                        stop=True,
                    )
                ht = hp.tile([P, ROWS_T], f32r, tag=f"h0_{m}", name=f"h0_{m}")
                nc.scalar.activation(ht[:], ps[:], Tanh, bias=bts[:, m : m + 1])
                return ht

            def l4_half(r, hf, ot):
                c = hf * 512
                psl = pl.tile([1, 512], f32, tag="psl", name=f"psl{hf}")
                h3 = hs[(3, r)]
                for j in range(KT):
                    nc.tensor.matmul(
                        psl[:],
                        wls[:, j : j + 1],
                        h3[j][:, c : c + 512],
                        start=(j == 0),
                        stop=(j == KT - 1),
                    )
                nc.vector.tensor_copy(ot[:, c : c + 512], psl[:])

            # prologue: layer 0 of row-tile 0
            hs[(0, 0)] = [l0_group(0, m) for m in range(KT)]

            ots = {}
            for r in range(N_RT + 1):
                # phase P1(r): L1(r) interleaved with L3(r-1)
                if r < N_RT:
                    hs[(1, r)] = [None] * KT
                if r >= 1:
                    hs[(3, r - 1)] = [None] * KT
                for m in range(KT):
                    if r < N_RT:
                        hs[(1, r)][m] = dense_group(
                            r, 1, m, w1s, hs[(0, r)], "h1", list(range(KT)), m * P
                        )
                    if r >= 1:
                        hs[(3, r - 1)][m] = dense_group(
                            r - 1, 3, m, w3s, hs[(2, r - 1)],
                            "h3", [(m // 2) * 2 + j for j in range(2)], ((m % 2) * P),
                        )
                if r >= 1:
                    hs.pop((0, r - 1), None)
                    hs.pop((2, r - 1), None)

                # phase P2(r): L2(r) + L0(r+1) + L4(r-1)
                if r < N_RT:
                    hs[(2, r)] = [None] * KT
                    load_x(r + 2)
                if r + 1 < N_RT:
                    hs[(0, r + 1)] = [None] * KT
                if r >= 1:
                    ots[r - 1] = op.tile([1, ROWS_T], f32, tag="o", name=f"ot{r-1}")
                for m in range(KT):
                    if r < N_RT:
                        hs[(2, r)][m] = dense_group(
                            r, 2, m, w2s, hs[(1, r)],
                            "h2", [(m // 4) * 4 + j for j in range(4)], ((m % 4) * P),
                        )
                    if r + 1 < N_RT:
                        hs[(0, r + 1)][m] = l0_group(r + 1, m)
                    if r >= 1 and m == 2:
                        l4_half(r - 1, 0, ots[r - 1])
                    if r >= 1 and m == 5:
                        l4_half(r - 1, 1, ots[r - 1])
                if r >= 1:
                    rs = (r - 1) * ROWS_T
                    nc.sync.dma_start(outT[:, rs : rs + ROWS_T], ots[r - 1][:])
                    hs.pop((3, r - 1), None)
                if r >= 1:
                    hs.pop((1, r - 1), None)

    nc.compile()
    return nc


def _get_module():
    if "nc" not in _STATE:
        _STATE["nc"] = _build_module()
    return _STATE["nc"]


def _tcol(v):
    """[1024] vector -> [128, 8]: column t holds v[t*128:(t+1)*128]."""
    return np.ascontiguousarray(v.reshape(KT, P).T).astype(np.float32)


def prep_in_maps(inputs):
    X = np.asarray(inputs["X"], np.float32)
    W0 = np.asarray(inputs["W0"], np.float32)
    b0 = np.asarray(inputs["b0"], np.float32)
    W1 = np.asarray(inputs["W1"], np.float32)
    b1 = np.asarray(inputs["b1"], np.float32)
    W2 = np.asarray(inputs["W2"], np.float32)
    b2 = np.asarray(inputs["b2"], np.float32)
    W3 = np.asarray(inputs["W3"], np.float32)
    b3 = np.asarray(inputs["b3"], np.float32)
    Wl = np.asarray(inputs["W_last"], np.float32)

    # fold `h = 2*(X-LB)/(UB-LB) - 1` (LB=[0,0], UB=[2pi,1]) into layer 0
    s = np.array([1.0 / np.pi, 2.0], np.float32)
    w0p = np.ascontiguousarray(s[:, None] * W0)
    b0p = b0[0] - W0[0] - W0[1]

    # pack only the nonzero 128-blocks of the masked layers
    w2p = np.ascontiguousarray(
        np.concatenate(
            [W2[k * P : (k + 1) * P, (k // 4) * 512 : (k // 4) * 512 + 512] for k in range(KT)],
            axis=0,
        )
    )
    w3p = np.ascontiguousarray(
        np.concatenate(
            [W3[k * P : (k + 1) * P, (k // 2) * 256 : (k // 2) * 256 + 256] for k in range(KT)],
            axis=0,
        )
    )
    wlp = _tcol(Wl[:, 0])
    btp = np.ascontiguousarray(
        np.concatenate([_tcol(b0p), _tcol(b1[0]), _tcol(b2[0]), _tcol(b3[0])], axis=1)
    )

    xT = np.ascontiguousarray(X.T)  # [2, 65536]
    common = {"w0": w0p, "w1": W1, "w2": w2p, "w3": w3p, "wl": wlp, "bt": btp}
    return [
        {"xT": np.ascontiguousarray(xT[:, c * R : (c + 1) * R]), **common}
        for c in range(N_CORES)
    ]


def kernel(**inputs):
    from concourse.bass_utils import run_bass_kernel_spmd

    nc = _get_module()
    in_maps = prep_in_maps(inputs)
    trace = bool(int(os.environ.get("BASS_KERNEL_TRACE", "0")))
    res = run_bass_kernel_spmd(nc, in_maps, list(range(N_CORES)), trace=trace)
    _STATE["last_result"] = res
    out = np.concatenate([res.results[c]["outT"].reshape(-1) for c in range(N_CORES)])
    b_last = np.asarray(inputs["b_last"], np.float32)
    return (out.reshape(-1, 1) + b_last).astype(np.float32)


# revision 6
# speedup vs baseline: 1.1130x; 1.0363x over previous
"""BsPINN forward MLP on 8 Trainium2 NeuronCores (data-parallel over batch).

Network (reference): X[65536,2] -> normalize -> tanh(XW0+b0) -> tanh(hW1+b1)
  -> tanh(h(W2*mask2)+b2) -> tanh(h(W3*mask3)+b3) -> hW_last+b_last -> [65536,1]
mask2 = blockdiag(2 x [512x512] ones), mask3 = blockdiag(4 x [256x256] ones).

Device strategy (per core, 8192 rows, row-tiles of 1024):
  - Activations kept TRANSPOSED in SBUF: [features(partitions), rows(free)].
    Dense layers are psum[m] += W[kblk,mblk].T @ hT[kblk] on the PE
    (out = lhsT.T @ rhs), float32r (full-rate), no transposes anywhere.
  - Input normalization folded into W0/b0 on the host; layer 0 (K=2) runs on
    the idle Vector engine (x broadcast via 0-stride DMA) instead of wasting
    the 128x128 PE array on K=2 matmuls.
  - Masked layers packed on host to only their nonzero 128-blocks
    (layer2: 4 k-blocks per m-tile, layer3: 2).
  - bias+tanh fused into one ScalarE activation per [128,1024] psum pair.
  - Two-phase software pipeline across row-tiles so matmul-dense work always
    overlaps the tanh-heavy psum drains (keeps PE busy and its clock warm):
      P1(r): L1(r) m0..6 interleaved with L3(r-1)
      P2(r): L1(r) m7, L2(r), L4(r-1), and layer-0(r+1) on DVE
"""

import os

import numpy as np

N_CORES = 8
N_ROWS = 65536
R = N_ROWS // N_CORES  # rows per core
ROWS_T = 1024  # rows per row-tile (psum pair = 2 banks per m-tile)
N_RT = R // ROWS_T
H = 1024
P = 128
KT = H // P  # 8 feature tiles

_STATE = {}


def _build_module():
    import concourse.bacc as bacc
    import concourse.mybir as mybir
    import concourse.tile as tile

    f32 = mybir.dt.float32
    f32r = mybir.dt.float32r
    Tanh = mybir.ActivationFunctionType.Tanh

    nc = bacc.Bacc("TRN2", target_bir_lowering=False, debug=False)

    xT = nc.dram_tensor("xT", [2, R], f32, kind="ExternalInput")
    w0 = nc.dram_tensor("w0", [P, 2 * KT], f32, kind="ExternalInput")
    w1 = nc.dram_tensor("w1", [H, H], f32r, kind="ExternalInput")
    w2 = nc.dram_tensor("w2", [H, 512], f32r, kind="ExternalInput")
    w3 = nc.dram_tensor("w3", [H, 256], f32r, kind="ExternalInput")
    wl = nc.dram_tensor("wl", [P, KT], f32r, kind="ExternalInput")
    bt = nc.dram_tensor("bt", [P, 4 * KT], f32, kind="ExternalInput")
    outT = nc.dram_tensor("outT", [1, R], f32, kind="ExternalOutput")

    with tile.TileContext(nc) as tc:
        with (
            tc.tile_pool(name="wpool", bufs=1) as wp,
            tc.tile_pool(name="hpool", bufs=1) as hp,
            tc.tile_pool(name="xpool", bufs=1) as xp,
            tc.tile_pool(name="opool", bufs=1) as op,
            tc.tile_pool(name="psum", bufs=4, space="PSUM") as pp,
        ):
            bts = wp.tile([P, 4 * KT], f32, tag="bt")
            w0s = wp.tile([P, 2 * KT], f32, tag="w0")
            nc.sync.dma_start(bts[:], bt[:])
            nc.sync.dma_start(w0s[:], w0[:])

            xbs = [None] * (N_RT + 1)

            def load_xb(r):
                if r < N_RT:
                    rs = r * ROWS_T
                    b0 = xp.tile([P, ROWS_T], f32, tag="xb0", name=f"xb0_{r}")
                    b1 = xp.tile([P, ROWS_T], f32, tag="xb1", name=f"xb1_{r}")
                    nc.sync.dma_start(b0[:], xT[0, rs : rs + ROWS_T].partition_broadcast(P))
                    nc.sync.dma_start(b1[:], xT[1, rs : rs + ROWS_T].partition_broadcast(P))
                    xbs[r] = (b0, b1)

            load_xb(0)
            w1s = []
            w2s = []
            w3s = []
            for k in range(KT):
                t = wp.tile([P, H], f32r, tag=f"w1_{k}")
                nc.sync.dma_start(t[:], w1[k * P : (k + 1) * P, :])
                w1s.append(t)
            for k in range(KT):
                t = wp.tile([P, 512], f32r, tag=f"w2_{k}")
                nc.sync.dma_start(t[:], w2[k * P : (k + 1) * P, :])
                w2s.append(t)
            for k in range(KT):
                t = wp.tile([P, 256], f32r, tag=f"w3_{k}")
                nc.sync.dma_start(t[:], w3[k * P : (k + 1) * P, :])
                w3s.append(t)
            wls = wp.tile([P, KT], f32r, tag="wl")
            nc.sync.dma_start(wls[:], wl[:])

            hs = {}  # (layer, r) -> list of 8 tiles

            def dense_group(layer, m, wk, hin, hout_tag, ks, mo):
                """One m-tile group: PE matmuls over ks into a psum pair + tanh."""
                ps = pp.tile([P, ROWS_T], f32, tag="ps", name=f"ps{layer}_{m}")
                for hf in range(2):
                    c = hf * 512
                    for j, k in enumerate(ks):
                        nc.tensor.matmul(
                            ps[:, c : c + 512],
                            wk[k][:, mo(k) : mo(k) + P] if callable(mo) else wk[k][:, mo : mo + P],
                            hin[k][:, c : c + 512],
                            start=(j == 0),
                            stop=(j == len(ks) - 1),
                        )
                ht = hp.tile([P, ROWS_T], f32r, tag=f"{hout_tag}_{m}", name=f"{hout_tag}_{m}")
                nc.scalar.activation(
                    ht[:], ps[:], Tanh, bias=bts[:, layer * KT + m : layer * KT + m + 1]
                )
                return ht

            def l0_group(r, m):
                """Layer 0 on DVE: h0[m] = tanh(x0*w0c0[m] + x1*w0c1[m] + b0[m])."""
                b0, b1 = xbs[r]
                t1 = hp.tile([P, ROWS_T], f32, tag="t1", name=f"t1_{m}")
                t2 = hp.tile([P, ROWS_T], f32, tag="t2", name=f"t2_{m}")
                nc.vector.tensor_scalar_mul(t1[:], b0[:], w0s[:, m : m + 1])
                nc.vector.tensor_scalar_mul(t2[:], b1[:], w0s[:, KT + m : KT + m + 1])
                nc.vector.tensor_add(t1[:], t1[:], t2[:])
                ht = hp.tile([P, ROWS_T], f32r, tag=f"h0_{m}", name=f"h0_{m}")
                nc.scalar.activation(ht[:], t1[:], Tanh, bias=bts[:, m : m + 1])
                return ht

            def l4_half(r, hf, ot):
                c = hf * 512
                psl = pp.tile([1, 512], f32, tag="ps", name=f"psl{hf}")
                h3 = hs[(3, r)]
                for j in range(KT):
                    nc.tensor.matmul(
                        psl[:],
                        wls[:, j : j + 1],
                        h3[j][:, c : c + 512],
                        start=(j == 0),
                        stop=(j == KT - 1),
                    )
                nc.vector.tensor_copy(ot[:, c : c + 512], psl[:])

            # prologue: layer 0 of row-tile 0 on DVE
            hs[(0, 0)] = [l0_group(0, m) for m in range(KT)]

            ots = {}
            for r in range(N_RT + 1):
                # ---- phase P1(r): L1(r) m0..6 interleaved with L3(r-1) ----
                load_xb(r + 1)
                if r < N_RT:
                    hs[(1, r)] = [None] * KT
                if r >= 1:
                    hs[(3, r - 1)] = [None] * KT
                for m in range(KT):
                    if r < N_RT and m < KT - 1:
                        hs[(1, r)][m] = dense_group(
                            1, m, w1s, hs[(0, r)], "h1", list(range(KT)), m * P
                        )
                    if r >= 1:
                        hs[(3, r - 1)][m] = dense_group(
                            3, m, w3s, hs[(2, r - 1)],
                            "h3", [(m // 2) * 2 + j for j in range(2)], ((m % 2) * P),
                        )
                if r >= 1:
                    hs.pop((2, r - 1), None)

                # ---- phase P2(r): L1(r) m7, L2(r), L4(r-1), L0(r+1) on DVE ----
                if r < N_RT:
                    hs[(2, r)] = [None] * KT
                    hs[(1, r)][KT - 1] = dense_group(
                        1, KT - 1, w1s, hs[(0, r)], "h1", list(range(KT)), (KT - 1) * P
                    )
                if r >= 1:
                    hs.pop((0, r - 1), None)
                if r + 1 < N_RT:
                    hs[(0, r + 1)] = [None] * KT
                if r >= 1:
                    ots[r - 1] = op.tile([1, ROWS_T], f32, tag="o", name=f"ot{r-1}")
                for m in range(KT):
                    if r < N_RT:
                        hs[(2, r)][m] = dense_group(
                            2, m, w2s, hs[(1, r)],
                            "h2", [(m // 4) * 4 + j for j in range(4)], ((m % 4) * P),
                        )
                    if r + 1 < N_RT:
                        hs[(0, r + 1)][m] = l0_group(r + 1, m)
                    if r >= 1 and m == 1:
                        l4_half(r - 1, 0, ots[r - 1])
                    if r >= 1 and m == 4:
                        l4_half(r - 1, 1, ots[r - 1])
                if r >= 1:
                    rs = (r - 1) * ROWS_T
                    nc.sync.dma_start(outT[:, rs : rs + ROWS_T], ots[r - 1][:])
                    hs.pop((3, r - 1), None)
                if r >= 1:
                    hs.pop((1, r - 1), None)

    nc.compile()
    return nc


def _get_module():
    if "nc" not in _STATE:
        _STATE["nc"] = _build_module()
    return _STATE["nc"]


def _tcol(v):
    """[1024] vector -> [128, 8]: column t holds v[t*128:(t+1)*128]."""
    return np.ascontiguousarray(v.reshape(KT, P).T).astype(np.float32)


def prep_in_maps(inputs):
    X = np.asarray(inputs["X"], np.float32)
    W0 = np.asarray(inputs["W0"], np.float32)
    b0 = np.asarray(inputs["b0"], np.float32)
    W1 = np.asarray(inputs["W1"], np.float32)
    b1 = np.asarray(inputs["b1"], np.float32)
    W2 = np.asarray(inputs["W2"], np.float32)
    b2 = np.asarray(inputs["b2"], np.float32)
    W3 = np.asarray(inputs["W3"], np.float32)
    b3 = np.asarray(inputs["b3"], np.float32)
    Wl = np.asarray(inputs["W_last"], np.float32)

    # fold `h = 2*(X-LB)/(UB-LB) - 1` (LB=[0,0], UB=[2pi,1]) into layer 0
    s = np.array([1.0 / np.pi, 2.0], np.float32)
    w0p = s[:, None] * W0
    b0p = b0[0] - W0[0] - W0[1]
    # layer-0 weights in per-partition layout for DVE: [128, 16]
    w0t = np.ascontiguousarray(np.concatenate([_tcol(w0p[0]), _tcol(w0p[1])], axis=1))

    # pack only the nonzero 128-blocks of the masked layers
    w2p = np.ascontiguousarray(
        np.concatenate(
            [W2[k * P : (k + 1) * P, (k // 4) * 512 : (k // 4) * 512 + 512] for k in range(KT)],
            axis=0,
        )
    )
    w3p = np.ascontiguousarray(
        np.concatenate(
            [W3[k * P : (k + 1) * P, (k // 2) * 256 : (k // 2) * 256 + 256] for k in range(KT)],
            axis=0,
        )
    )
    wlp = _tcol(Wl[:, 0])
    btp = np.ascontiguousarray(
        np.concatenate([_tcol(b0p), _tcol(b1[0]), _tcol(b2[0]), _tcol(b3[0])], axis=1)
    )

    xT = np.ascontiguousarray(X.T)  # [2, 65536]
    common = {"w0": w0t, "w1": W1, "w2": w2p, "w3": w3p, "wl": wlp, "bt": btp}
    return [
        {"xT": np.ascontiguousarray(xT[:, c * R : (c + 1) * R]), **common}
        for c in range(N_CORES)
    ]


def kernel(**inputs):
    from concourse.bass_utils import run_bass_kernel_spmd

    nc = _get_module()
    in_maps = prep_in_maps(inputs)
    trace = bool(int(os.environ.get("BASS_KERNEL_TRACE", "0")))
    res = run_bass_kernel_spmd(nc, in_maps, list(range(N_CORES)), trace=trace)
    _STATE["last_result"] = res
    out = np.concatenate([res.results[c]["outT"].reshape(-1) for c in range(N_CORES)])
    b_last = np.asarray(inputs["b_last"], np.float32)
    return (out.reshape(-1, 1) + b_last).astype(np.float32)


# revision 8
# speedup vs baseline: 1.1638x; 1.0456x over previous
"""BsPINN forward MLP on 8 Trainium2 NeuronCores (data-parallel over batch).

Network (reference): X[65536,2] -> normalize -> tanh(XW0+b0) -> tanh(hW1+b1)
  -> tanh(h(W2*mask2)+b2) -> tanh(h(W3*mask3)+b3) -> hW_last+b_last -> [65536,1]
mask2 = blockdiag(2 x [512x512] ones), mask3 = blockdiag(4 x [256x256] ones).

Device strategy (per core, 8192 rows, row-tiles of 1024):
  - Activations kept TRANSPOSED in SBUF: [features(partitions), rows(free)].
    Dense layers are psum[m] += W[kblk,mblk].T @ hT[kblk] on the PE
    (out = lhsT.T @ rhs), float32r (full-rate), no transposes anywhere.
  - Input normalization folded into W0/b0 on the host; layer 0 (K=2) runs on
    the idle Vector engine (x broadcast via 0-stride DMA) instead of wasting
    the 128x128 PE array on K=2 matmuls.
  - Masked layers packed on host to only their nonzero 128-blocks
    (layer2: 4 k-blocks per m-tile, layer3: 2).
  - bias+tanh fused into one ScalarE activation per [128,1024] psum pair.
  - Two-phase software pipeline across row-tiles so matmul-dense work always
    overlaps the tanh-heavy psum drains (keeps PE busy and its clock warm):
      P1(r): L1(r) m0..6 interleaved with L3(r-1)
      P2(r): L1(r) m7, L2(r), L4(r-1), and layer-0(r+1) on DVE
"""

import os

import numpy as np

N_CORES = 8
N_ROWS = 65536
R = N_ROWS // N_CORES  # rows per core
ROWS_T = 1024  # rows per row-tile (psum pair = 2 banks per m-tile)
N_RT = R // ROWS_T
H = 1024
P = 128
KT = H // P  # 8 feature tiles

_STATE = {}


def _build_module():
    import concourse.bacc as bacc
    import concourse.mybir as mybir
    import concourse.tile as tile

    f32 = mybir.dt.float32
    f32r = mybir.dt.float32r
    Tanh = mybir.ActivationFunctionType.Tanh

    nc = bacc.Bacc("TRN2", target_bir_lowering=False, debug=False)

    xT = nc.dram_tensor("xT", [2, R], f32, kind="ExternalInput")
    w0 = nc.dram_tensor("w0", [P, 2 * KT], f32, kind="ExternalInput")
    w1 = nc.dram_tensor("w1", [H, H], f32r, kind="ExternalInput")
    w2 = nc.dram_tensor("w2", [H, 512], f32r, kind="ExternalInput")
    w3 = nc.dram_tensor("w3", [H, 256], f32r, kind="ExternalInput")
    wl = nc.dram_tensor("wl", [P, KT], f32r, kind="ExternalInput")
    bt = nc.dram_tensor("bt", [P, 4 * KT], f32, kind="ExternalInput")
    x0r = nc.dram_tensor("x0r", [2, ROWS_T], f32r, kind="ExternalInput")
    w0k = nc.dram_tensor("w0k", [2, H], f32r, kind="ExternalInput")
    outT = nc.dram_tensor("outT", [1, R], f32, kind="ExternalOutput")

    with tile.TileContext(nc) as tc:
        with (
            tc.tile_pool(name="wpool", bufs=1) as wp,
            tc.tile_pool(name="hpool", bufs=1) as hp,
            tc.tile_pool(name="xpool", bufs=1) as xp,
            tc.tile_pool(name="opool", bufs=1) as op,
            tc.tile_pool(name="psum", bufs=4, space="PSUM") as pp,
        ):
            bts = wp.tile([P, 4 * KT], f32, tag="bt")
            w0s = wp.tile([P, 2 * KT], f32, tag="w0")
            nc.sync.dma_start(bts[:], bt[:])
            nc.sync.dma_start(w0s[:], w0[:])

            xbs = [None] * (N_RT + 1)

            def load_xb(r):
                if r < N_RT:
                    rs = r * ROWS_T
                    b0 = xp.tile([P, ROWS_T], f32, tag="xb0", name=f"xb0_{r}")
                    b1 = xp.tile([P, ROWS_T], f32, tag="xb1", name=f"xb1_{r}")
                    nc.gpsimd.dma_start(b0[:], xT[0, rs : rs + ROWS_T].partition_broadcast(P))
                    nc.gpsimd.dma_start(b1[:], xT[1, rs : rs + ROWS_T].partition_broadcast(P))
                    xbs[r] = (b0, b1)

            xs0t = hp.tile([P, ROWS_T], f32, tag="t1", name="xs0boot")
            w0kt = hp.tile([P, ROWS_T], f32, tag="t2", name="w0kboot")
            nc.sync.dma_start(xs0t[0:2, :].bitcast(f32r), x0r[:])
            nc.sync.dma_start(w0kt[0:2, :].bitcast(f32r), w0k[:])
            load_xb(0)
            w1s = []
            w2s = []
            w3s = []
            for k in range(KT):
                t = wp.tile([P, H], f32r, tag=f"w1_{k}")
                nc.sync.dma_start(t[:], w1[k * P : (k + 1) * P, :])
                w1s.append(t)
            for k in range(KT):
                t = wp.tile([P, 512], f32r, tag=f"w2_{k}")
                nc.sync.dma_start(t[:], w2[k * P : (k + 1) * P, :])
                w2s.append(t)
            for k in range(KT):
                t = wp.tile([P, 256], f32r, tag=f"w3_{k}")
                nc.sync.dma_start(t[:], w3[k * P : (k + 1) * P, :])
                w3s.append(t)
            wls = wp.tile([P, KT], f32r, tag="wl")
            nc.sync.dma_start(wls[:], wl[:])

            hs = {}  # (layer, r) -> list of 8 tiles

            def dense_group(layer, m, wk, hin, hout_tag, ks, mo):
                """One m-tile group: PE matmuls over ks into a psum pair + tanh."""
                ps = pp.tile([P, ROWS_T], f32, tag="ps", name=f"ps{layer}_{m}")
                for hf in range(2):
                    c = hf * 512
                    for j, k in enumerate(ks):
                        nc.tensor.matmul(
                            ps[:, c : c + 512],
                            wk[k][:, mo(k) : mo(k) + P] if callable(mo) else wk[k][:, mo : mo + P],
                            hin[k][:, c : c + 512],
                            start=(j == 0),
                            stop=(j == len(ks) - 1),
                        )
                ht = hp.tile([P, ROWS_T], f32r, tag=f"{hout_tag}_{m}", name=f"{hout_tag}_{m}")
                nc.scalar.activation(
                    ht[:], ps[:], Tanh, bias=bts[:, layer * KT + m : layer * KT + m + 1]
                )
                return ht

            def l0_group(r, m):
                """Layer 0 on DVE: h0[m] = tanh(x0*w0c0[m] + x1*w0c1[m] + b0[m])."""
                b0, b1 = xbs[r]
                t1 = hp.tile([P, ROWS_T], f32, tag="t1", name=f"t1_{m}")
                t2 = hp.tile([P, ROWS_T], f32, tag="t2", name=f"t2_{m}")
                nc.vector.tensor_scalar_mul(t1[:], b0[:], w0s[:, m : m + 1])
                nc.vector.tensor_scalar_mul(t2[:], b1[:], w0s[:, KT + m : KT + m + 1])
                nc.vector.tensor_add(t1[:], t1[:], t2[:])
                ht = hp.tile([P, ROWS_T], f32r, tag=f"h0_{m}", name=f"h0_{m}")
                nc.scalar.activation(ht[:], t1[:], Tanh, bias=bts[:, m : m + 1])
                return ht

            def l4_half(r, hf, ot):
                c = hf * 512
                psl = pp.tile([1, 512], f32, tag="ps", name=f"psl{hf}")
                h3 = hs[(3, r)]
                for j in range(KT):
                    nc.tensor.matmul(
                        psl[:],
                        wls[:, j : j + 1],
                        h3[j][:, c : c + 512],
                        start=(j == 0),
                        stop=(j == KT - 1),
                    )
                nc.vector.tensor_copy(ot[0:1, c : c + 512], psl[:])

            # prologue: layer 0 of row-tile 0 on the PE (fast startup; the
            # steady-state layer 0 runs on DVE via l0_group)
            def l0_pe_group(m):
                ps = pp.tile([P, ROWS_T], f32, tag="ps", name=f"psb_{m}")
                for hf in range(2):
                    c = hf * 512
                    nc.tensor.matmul(
                        ps[:, c : c + 512],
                        w0kt[0:2, m * P : (m + 1) * P].bitcast(f32r),
                        xs0t[0:2, c : c + 512].bitcast(f32r),
                        start=True,
                        stop=True,
                    )
                ht = hp.tile([P, ROWS_T], f32r, tag=f"h0_{m}", name=f"h0_{m}")
                nc.scalar.activation(ht[:], ps[:], Tanh, bias=bts[:, m : m + 1])
                return ht

            hs[(0, 0)] = [l0_pe_group(m) for m in range(KT)]

            ots = {}
            for r in range(N_RT + 1):
                # ---- phase P1(r): L1(r) m0..6 interleaved with L3(r-1) ----
                load_xb(r + 1)
                if r < N_RT:
                    hs[(1, r)] = [None] * KT
                if r >= 1:
                    hs[(3, r - 1)] = [None] * KT
                for m in range(KT):
                    if r < N_RT and m < KT - 1:
                        hs[(1, r)][m] = dense_group(
                            1, m, w1s, hs[(0, r)], "h1", list(range(KT)), m * P
                        )
                    if r >= 1:
                        hs[(3, r - 1)][m] = dense_group(
                            3, m, w3s, hs[(2, r - 1)],
                            "h3", [(m // 2) * 2 + j for j in range(2)], ((m % 2) * P),
                        )
                if r >= 1:
                    hs.pop((2, r - 1), None)

                # ---- phase P2(r): L1(r) m7, L2(r), L4(r-1), L0(r+1) on DVE ----
                if r < N_RT:
                    hs[(2, r)] = [None] * KT
                    hs[(1, r)][KT - 1] = dense_group(
                        1, KT - 1, w1s, hs[(0, r)], "h1", list(range(KT)), (KT - 1) * P
                    )
                if r >= 1:
                    hs.pop((0, r - 1), None)
                if r + 1 < N_RT:
                    hs[(0, r + 1)] = [None] * KT
                if r >= 1:
                    ots[r - 1] = op.tile([P, ROWS_T], f32, tag="o", name=f"ot{r-1}")
                for m in range(KT):
                    if r < N_RT:
                        hs[(2, r)][m] = dense_group(
                            2, m, w2s, hs[(1, r)],
                            "h2", [(m // 4) * 4 + j for j in range(4)], ((m % 4) * P),
                        )
                    if r + 1 < N_RT:
                        hs[(0, r + 1)][m] = l0_group(r + 1, m)
                    if r >= 1 and m == 1:
                        l4_half(r - 1, 0, ots[r - 1])
                    if r >= 1 and m == 4:
                        l4_half(r - 1, 1, ots[r - 1])
                if r >= 1:
                    rs = (r - 1) * ROWS_T
                    nc.sync.dma_start(outT[:, rs : rs + ROWS_T], ots[r - 1][0:1, :])
                    hs.pop((3, r - 1), None)
                if r >= 1:
                    hs.pop((1, r - 1), None)

    nc.compile()
    return nc


def _get_module():
    if "nc" not in _STATE:
        _STATE["nc"] = _build_module()
    return _STATE["nc"]


def _tcol(v):
    """[1024] vector -> [128, 8]: column t holds v[t*128:(t+1)*128]."""
    return np.ascontiguousarray(v.reshape(KT, P).T).astype(np.float32)


def prep_in_maps(inputs):
    X = np.asarray(inputs["X"], np.float32)
    W0 = np.asarray(inputs["W0"], np.float32)
    b0 = np.asarray(inputs["b0"], np.float32)
    W1 = np.asarray(inputs["W1"], np.float32)
    b1 = np.asarray(inputs["b1"], np.float32)
    W2 = np.asarray(inputs["W2"], np.float32)
    b2 = np.asarray(inputs["b2"], np.float32)
    W3 = np.asarray(inputs["W3"], np.float32)
    b3 = np.asarray(inputs["b3"], np.float32)
    Wl = np.asarray(inputs["W_last"], np.float32)

    # fold `h = 2*(X-LB)/(UB-LB) - 1` (LB=[0,0], UB=[2pi,1]) into layer 0
    s = np.array([1.0 / np.pi, 2.0], np.float32)
    w0p = s[:, None] * W0
    b0p = b0[0] - W0[0] - W0[1]
    # layer-0 weights in per-partition layout for DVE: [128, 16]
    w0t = np.ascontiguousarray(np.concatenate([_tcol(w0p[0]), _tcol(w0p[1])], axis=1))

    # pack only the nonzero 128-blocks of the masked layers
    w2p = np.ascontiguousarray(
        np.concatenate(
            [W2[k * P : (k + 1) * P, (k // 4) * 512 : (k // 4) * 512 + 512] for k in range(KT)],
            axis=0,
        )
    )
    w3p = np.ascontiguousarray(
        np.concatenate(
            [W3[k * P : (k + 1) * P, (k // 2) * 256 : (k // 2) * 256 + 256] for k in range(KT)],
            axis=0,
        )
    )
    wlp = _tcol(Wl[:, 0])
    btp = np.ascontiguousarray(
        np.concatenate([_tcol(b0p), _tcol(b1[0]), _tcol(b2[0]), _tcol(b3[0])], axis=1)
    )

    xT = np.ascontiguousarray(X.T)  # [2, 65536]
    common = {
        "w0": w0t, "w1": W1, "w2": w2p, "w3": w3p, "wl": wlp, "bt": btp,
        "w0k": np.ascontiguousarray(w0p),
    }
    return [
        {
            "xT": np.ascontiguousarray(xT[:, c * R : (c + 1) * R]),
            "x0r": np.ascontiguousarray(xT[:, c * R : c * R + ROWS_T]),
            **common,
        }
        for c in range(N_CORES)
    ]


def kernel(**inputs):
    from concourse.bass_utils import run_bass_kernel_spmd

    nc = _get_module()
    in_maps = prep_in_maps(inputs)
    trace = bool(int(os.environ.get("BASS_KERNEL_TRACE", "0")))
    res = run_bass_kernel_spmd(nc, in_maps, list(range(N_CORES)), trace=trace)
    _STATE["last_result"] = res
    out = np.concatenate([res.results[c]["outT"].reshape(-1) for c in range(N_CORES)])
    b_last = np.asarray(inputs["b_last"], np.float32)
    return (out.reshape(-1, 1) + b_last).astype(np.float32)
